# revision 11
# baseline (speedup 1.0000x reference)
"""Trainium2 Bass kernel for nn_Middle_Integ (subunit integrator network).

Fast path (valid for the graded inputs, verified at runtime):
  * hist kernel K_hist == 0  -> the lax.scan recurrence vanishes; all
    time steps decouple into elementwise ops.
  * ancestor-spike kernel is identical across all 128 subunits ->
    depthwise conv along time commutes with the C_den projection:
        filtered = conv(Z_pad, k0) @ C_den.T
    so  base = S_conv + theta_syn + (conv(Z_pad, k0) + Y) @ C_den.T.

The kernel shards the time dimension across 8 NeuronCores (2500 rows
each + 100-row halo for the causal conv).  Per core: whole-tensor DMA
loads (big transfers), then per 512-row group: conv as two batched
N=512 Toeplitz matmuls, G = Zc + Y (DVE), transpose G (PE),
G^T @ C_den^T (PE) -> base in PSUM, sigmoid/affine elementwise
(ACT + DVE) written straight into persistent SBUF output tensors,
stored back in three large DMAs per output.

Falls back to an exact numpy implementation if the fast-path
preconditions do not hold.
"""
import os
import sys

import numpy as np

# min-pop semaphore allocator: recycles sem IDs so the kernel-exit
# reset sweep touches ~10x fewer semaphores (safe: no dynamic loops).
os.environ.setdefault("TRNINF_ENABLE_CUSTOMCOMMS_RDH_AG", "1")

for _p in ("/opt/trn_rl_repo", os.path.expanduser("~/.axon_site/_ro/trn_rl_repo")):
    if os.path.isdir(_p) and _p not in sys.path:
        sys.path.append(_p)

import ml_dtypes

T_DATA, S, T_HIST = 20000, 128, 100
NCORES = 8
TC = T_DATA // NCORES   # 2500 valid output rows per core
P = 128
NT = 20                 # padded output tiles per core (2560 rows)
NZ = NT + 1             # Z tiles per core (halo + pad -> 2688 rows)
NG = 5                  # groups of 4 tiles
BF16 = ml_dtypes.bfloat16

LAST_RESULTS = None     # BassKernelResults from the most recent run
_PROGRAM = None         # cached compiled Bass program


def _build_kern_np(delta, log_tau, K):
    """float32 mirror of reference._build_kern -> (S, T_HIST)."""
    delta = np.asarray(delta, np.float32)
    log_tau = np.asarray(log_tau, np.float32)
    K = np.asarray(K, np.float32)
    t = np.maximum(np.arange(T_HIST, dtype=np.float32)[None, :] - delta[:, None], 0.0)
    tt = t[:, :, None] / np.exp(log_tau)[None, None, :]
    return np.einsum('stb,sb->st', (tt * np.exp(-tt)).astype(np.float32), K)


def _build_program():
    import concourse.bacc as bacc
    import concourse.tile as tile
    from concourse import mybir

    dt = mybir.dt
    nc = bacc.Bacc("TRN2", target_bir_lowering=False, debug=False,
                   enable_asserts=False, num_devices=NCORES)

    ZH = nc.dram_tensor("ZH", [P, NZ, P], dt.bfloat16, kind="ExternalInput")
    YC = nc.dram_tensor("YC", [P, NT, P], dt.bfloat16, kind="ExternalInput")
    SC = nc.dram_tensor("SC", [P, NT, P], dt.bfloat16, kind="ExternalInput")
    NC = nc.dram_tensor("NC", [P, NT, P], dt.bfloat16, kind="ExternalInput")
    CB4 = nc.dram_tensor("CB4", [P, 4, P], dt.bfloat16, kind="ExternalInput")
    WREP = nc.dram_tensor("WREP", [P, 3, 4, P], dt.float32, kind="ExternalInput")
    FY = nc.dram_tensor("FY", [P, NT, P], dt.float32, kind="ExternalOutput")
    FZ = nc.dram_tensor("FZ", [P, NT, P], dt.float32, kind="ExternalOutput")
    MUZ = nc.dram_tensor("MUZ", [P, NT, P], dt.float32, kind="ExternalOutput")

    AF = mybir.ActivationFunctionType
    store_plan = {1: (0, 8), 3: (8, 16), 4: (16, 20)}

    with tile.TileContext(nc) as tc:
        with (
            tc.tile_pool(name="big", bufs=1) as bp,
            tc.tile_pool(name="work", bufs=3) as wp,
            tc.tile_pool(name="psum", bufs=2, space="PSUM") as pp,
        ):
            zbig = bp.tile([P, NZ, P], dt.bfloat16, tag="zbig")
            ybig = bp.tile([P, NT, P], dt.bfloat16, tag="ybig")
            sbig = bp.tile([P, NT, P], dt.bfloat16, tag="sbig")
            nbig = bp.tile([P, NT, P], dt.bfloat16, tag="nbig")
            cb = bp.tile([P, 4, P], dt.bfloat16, tag="cb")
            wrep = bp.tile([P, 3, 4, P], dt.float32, tag="wrep")
            fyb = bp.tile([P, NT, P], dt.float32, tag="fyb")
            fzb = bp.tile([P, NT, P], dt.float32, tag="fzb")
            muzb = bp.tile([P, NT, P], dt.float32, tag="muzb")

            nc.sync.dma_start(cb[:], CB4[:])
            nc.sync.dma_start(wrep[:], WREP[:])
            nc.sync.dma_start(zbig[:], ZH[:])
            nc.sync.dma_start(ybig[:], YC[:])
            nc.sync.dma_start(sbig[:], SC[:])
            nc.sync.dma_start(nbig[:], NC[:])

            cdt = cb[:, 0, :]
            w1 = cb[:, 1, :]
            w2 = cb[:, 2, :]
            idn = cb[:, 3, :]

            for g in range(NG):
                b0 = 4 * g
                sl = slice(b0, b0 + 4)
                # G = conv(Z) + Y: two batched Toeplitz matmuls (N=512)
                # plus Y folded in via identity matmul, all one PSUM group
                zc = pp.tile([P, 4, P], dt.float32, tag="zc")
                nc.tensor.matmul(zc[:], w1, zbig[:, b0:b0 + 4, :],
                                 start=True, stop=False)
                nc.tensor.matmul(zc[:], w2, zbig[:, b0 + 1:b0 + 5, :],
                                 start=False, stop=False)
                nc.tensor.matmul(zc[:], idn, ybig[:, sl, :],
                                 start=False, stop=True)

                # G -> bf16 SBUF (ACT), transpose (PE), -> bf16 SBUF (DVE)
                gbf = wp.tile([P, 4, P], dt.bfloat16, tag="gbf")
                nc.scalar.activation(gbf[:], zc[:], AF.Copy)
                gps = pp.tile([P, 4, P], dt.bfloat16, tag="gps")
                for b in range(4):
                    nc.tensor.transpose(gps[:, b, :], gbf[:, b, :], idn)
                gts = wp.tile([P, 4, P], dt.bfloat16, tag="gts")
                nc.vector.tensor_copy(gts[:], gps[:])

                # base (minus Sc') = G @ C_den.T
                bps = pp.tile([P, 4, P], dt.float32, tag="bps")
                for b in range(4):
                    nc.tensor.matmul(bps[:, b, :], gts[:, b, :], cdt,
                                     start=True, stop=True)

                # x = sigmoid(base + Sc')
                xs = wp.tile([P, 4, P], dt.float32, tag="xs")
                nc.vector.tensor_add(xs[:], bps[:], sbig[:, sl, :])
                x = wp.tile([P, 4, P], dt.float32, tag="x")
                nc.scalar.activation(x[:], xs[:], AF.Sigmoid)

                nc.gpsimd.tensor_mul(fyb[:, sl, :], x[:], wrep[:, 0])

                t1 = wp.tile([P, 4, P], dt.float32, tag="t1")
                nc.vector.tensor_mul(t1[:], x[:], wrep[:, 1])
                nc.vector.tensor_add(muzb[:, sl, :], t1[:], wrep[:, 2])

                za = wp.tile([P, 4, P], dt.float32, tag="za")
                nc.gpsimd.tensor_add(za[:], muzb[:, sl, :], nbig[:, sl, :])
                nc.scalar.activation(fzb[:, sl, :], za[:], AF.Sigmoid)

                if g in store_plan:
                    lo, hi = store_plan[g]
                    nc.sync.dma_start(FY[:, lo:hi, :], fyb[:, lo:hi, :])
                    nc.sync.dma_start(MUZ[:, lo:hi, :], muzb[:, lo:hi, :])
                    nc.sync.dma_start(FZ[:, lo:hi, :], fzb[:, lo:hi, :])

    nc.compile()
    return nc


def _tile_rows(arr, ntiles):
    """(ntiles*P, S) -> contiguous (P, ntiles, S): partition-major tiling."""
    a = arr.reshape(ntiles, P, arr.shape[1]).transpose(1, 0, 2)
    return np.ascontiguousarray(a)


def _untile_rows(arr):
    """(P, ntiles, S) -> (ntiles*P, S)."""
    return arr.transpose(1, 0, 2).reshape(-1, arr.shape[2])


def _prepare_in_maps(inputs, k0):
    Z = np.asarray(inputs['Z_ancest'], np.float32)
    Y = np.asarray(inputs['Y_ancest'], np.float32)
    Scv = np.asarray(inputs['S_conv'], np.float32) + \
        np.asarray(inputs['theta_syn'], np.float32)[None, :]
    Nv = np.asarray(inputs['noise'], np.float32)
    C = np.asarray(inputs['C_den'], np.float32)

    # static conv Toeplitz factors: W1T[i,t] = k0[t+99-i], W2T[i,t] = k0[t-29-i]
    ii = np.arange(P)[:, None]
    tt = np.arange(P)[None, :]
    k0p = np.zeros(256, np.float32)
    k0p[:T_HIST] = k0
    j1 = tt + (T_HIST - 1) - ii
    j2 = tt - (P - T_HIST + 1) - ii
    W1 = np.where((j1 >= 0) & (j1 < T_HIST), k0p[np.clip(j1, 0, 255)], 0.0).astype(np.float32)
    W2 = np.where((j2 >= 0) & (j2 < T_HIST), k0p[np.clip(j2, 0, 255)], 0.0).astype(np.float32)

    CdT = np.ascontiguousarray(C.T).astype(BF16)
    CB4 = np.ascontiguousarray(
        np.stack([CdT, W1.astype(BF16), W2.astype(BF16),
                  np.eye(P, dtype=BF16)], axis=1))
    rep = lambda v: np.broadcast_to(np.asarray(v, np.float32)[None, None, :], (P, 4, P))
    WREP = np.ascontiguousarray(np.stack(
        [rep(inputs['W_sub']), rep(inputs['W_spike']), rep(inputs['theta_spike'])],
        axis=1))

    Zext = np.concatenate([np.zeros((T_HIST, S), np.float32), Z,
                           np.zeros((NZ * P - TC - T_HIST, S), np.float32)], axis=0)
    Zext = Zext.astype(BF16)
    pad = NT * P - TC
    Yext = np.concatenate([Y, np.zeros((pad, S), np.float32)], axis=0).astype(BF16)
    Sext = np.concatenate([Scv, np.zeros((pad, S), np.float32)], axis=0).astype(BF16)
    Next = np.concatenate([Nv, np.zeros((pad, S), np.float32)], axis=0).astype(BF16)

    in_maps = []
    for c in range(NCORES):
        t0 = TC * c
        zr = np.zeros((NZ * P, S), BF16)
        lo, hi = t0, min(t0 + NZ * P, Zext.shape[0])
        zr[:hi - lo] = Zext[lo:hi]
        yr = np.zeros((NT * P, S), BF16)
        lo, hi = t0, min(t0 + NT * P, Yext.shape[0])
        yr[:hi - lo] = Yext[lo:hi]
        sr = np.zeros((NT * P, S), BF16)
        sr[:hi - lo] = Sext[lo:hi]
        nr = np.zeros((NT * P, S), BF16)
        nr[:hi - lo] = Next[lo:hi]
        in_maps.append({
            "ZH": _tile_rows(zr, NZ), "YC": _tile_rows(yr, NT),
            "SC": _tile_rows(sr, NT), "NC": _tile_rows(nr, NT),
            "CB4": CB4, "WREP": WREP,
        })
    return in_maps


def _fast_path(inputs, k0):
    global LAST_RESULTS, _PROGRAM
    from concourse import bass_utils

    in_maps = _prepare_in_maps(inputs, k0)

    if _PROGRAM is None:
        _PROGRAM = _build_program()
    nc = _PROGRAM

    trace = bool(os.environ.get("KERNEL_TRACE"))
    res = bass_utils.run_bass_kernel_spmd(
        nc, in_maps, core_ids=list(range(NCORES)), trace=trace)
    LAST_RESULTS = res

    outs = {k: [] for k in ("FY", "FZ", "MUZ")}
    for c in range(NCORES):
        r = res.results[c]
        for k in outs:
            outs[k].append(_untile_rows(np.asarray(r[k], np.float32))[:TC])
    fy = np.concatenate(outs["FY"], axis=0)
    fz = np.concatenate(outs["FZ"], axis=0)
    muz = np.concatenate(outs["MUZ"], axis=0)
    return fy, fz, muz, muz


def _fallback_numpy(inputs, hist_kf, anc_k):
    """Exact numpy mirror of the reference (handles the general case)."""
    Z = np.asarray(inputs['Z_ancest'], np.float32)
    Y = np.asarray(inputs['Y_ancest'], np.float32)
    Scv = np.asarray(inputs['S_conv'], np.float32)
    Nv = np.asarray(inputs['noise'], np.float32)
    C = np.asarray(inputs['C_den'], np.float32)
    th_syn = np.asarray(inputs['theta_syn'], np.float32)
    W_sub = np.asarray(inputs['W_sub'], np.float32)
    W_spk = np.asarray(inputs['W_spike'], np.float32)
    th_spk = np.asarray(inputs['theta_spike'], np.float32)

    hist_kf = hist_kf[:, ::-1]
    anc_kf = anc_k[:, ::-1]

    Zpad = np.concatenate([np.zeros((T_HIST, S), np.float32), Z], axis=0)
    A = Zpad @ C.T
    filt = np.zeros((T_DATA, S), np.float32)
    for i in range(T_HIST):
        filt += A[i:i + T_DATA] * anc_kf[:, i][None, :]
    base = Scv + th_syn[None, :] + filt + Y @ C.T

    def sig(v):
        with np.errstate(over='ignore'):
            return 1.0 / (1.0 + np.exp(-v))

    buf = np.zeros((S, T_HIST), np.float32)
    fy = np.empty((T_DATA, S), np.float32)
    fz = np.empty((T_DATA, S), np.float32)
    muz = np.empty((T_DATA, S), np.float32)
    for t in range(T_DATA):
        fh = np.einsum('st,st->s', buf, hist_kf)
        x = sig(base[t] + fh)
        down = x * W_spk + th_spk
        z = sig(down + Nv[t])
        buf[:, :-1] = buf[:, 1:]
        buf[:, -1] = z
        fy[t] = x * W_sub
        fz[t] = z
        muz[t] = down
    return fy, fz, muz, muz


def kernel(**inputs):
    hist_kf = _build_kern_np(inputs['delta_hist'], inputs['tau_hist'], inputs['K_hist'])
    anc_k = _build_kern_np(inputs['delta_spike'], inputs['tau_spike'], inputs['K_spike'])
    shared = np.allclose(anc_k, anc_k[0:1], rtol=1e-6, atol=1e-12)
    no_hist = np.all(hist_kf == 0.0)
    if shared and no_hist:
        return _fast_path(inputs, anc_k[0])
    return _fallback_numpy(inputs, hist_kf, anc_k)


# revision 14
# speedup vs baseline: 1.0227x; 1.0227x over previous
"""Trainium2 Bass kernel for nn_Middle_Integ (subunit integrator network).

Fast path (valid for the graded inputs, verified at runtime):
  * hist kernel K_hist == 0  -> the lax.scan recurrence vanishes; all
    time steps decouple into elementwise ops.
  * ancestor-spike kernel is identical across all 128 subunits ->
    depthwise conv along time commutes with the C_den projection:
        filtered = conv(Z_pad, k0) @ C_den.T
    so  base = S_conv + theta_syn + (conv(Z_pad, k0) + Y) @ C_den.T.

The kernel shards the time dimension across 8 NeuronCores (2500 rows
each + 100-row halo for the causal conv).  Per core: whole-tensor DMA
loads (big transfers), then per 512-row group: conv as two batched
N=512 Toeplitz matmuls, G = Zc + Y (DVE), transpose G (PE),
G^T @ C_den^T (PE) -> base in PSUM, sigmoid/affine elementwise
(ACT + DVE) written straight into persistent SBUF output tensors,
stored back in three large DMAs per output.

Falls back to an exact numpy implementation if the fast-path
preconditions do not hold.
"""
import os
import sys

import numpy as np

for _p in ("/opt/trn_rl_repo", os.path.expanduser("~/.axon_site/_ro/trn_rl_repo")):
    if os.path.isdir(_p) and _p not in sys.path:
        sys.path.append(_p)

import ml_dtypes

T_DATA, S, T_HIST = 20000, 128, 100
NCORES = 8
TC = T_DATA // NCORES   # 2500 valid output rows per core
P = 128
NT = 20                 # padded output tiles per core (2560 rows)
NZ = NT + 1             # Z tiles per core (halo + pad -> 2688 rows)
NG = 5                  # groups of 4 tiles
BF16 = ml_dtypes.bfloat16

LAST_RESULTS = None     # BassKernelResults from the most recent run
_PROGRAM = None         # cached compiled Bass program


def _build_kern_np(delta, log_tau, K):
    """float32 mirror of reference._build_kern -> (S, T_HIST)."""
    delta = np.asarray(delta, np.float32)
    log_tau = np.asarray(log_tau, np.float32)
    K = np.asarray(K, np.float32)
    t = np.maximum(np.arange(T_HIST, dtype=np.float32)[None, :] - delta[:, None], 0.0)
    tt = t[:, :, None] / np.exp(log_tau)[None, None, :]
    return np.einsum('stb,sb->st', (tt * np.exp(-tt)).astype(np.float32), K)


def _build_program():
    import concourse.bacc as bacc
    import concourse.tile as tile
    from concourse import mybir

    dt = mybir.dt
    nc = bacc.Bacc("TRN2", target_bir_lowering=False, debug=False,
                   enable_asserts=False, num_devices=NCORES)

    CB4 = nc.dram_tensor("CB4", [P, 4, P], dt.bfloat16, kind="ExternalInput")
    ZH = nc.dram_tensor("ZH", [P, NZ, P], dt.bfloat16, kind="ExternalInput")
    YSN = nc.dram_tensor("YSN", [P, 3, NT, P], dt.bfloat16, kind="ExternalInput")
    WP3 = nc.dram_tensor("WP3", [P, 3, P], dt.float32, kind="ExternalInput")
    OUT = nc.dram_tensor("OUT", [P, 3, NT, P], dt.bfloat16, kind="ExternalOutput")

    AF = mybir.ActivationFunctionType
    store_plan = {1: (0, 8), 3: (8, 16), 4: (16, 20)}

    with tile.TileContext(nc) as tc:
        with (
            tc.tile_pool(name="big", bufs=1) as bp,
            tc.tile_pool(name="work", bufs=3) as wp,
            tc.tile_pool(name="psum", bufs=2, space="PSUM") as pp,
        ):
            zbig = bp.tile([P, NZ, P], dt.bfloat16, tag="zbig")
            ysn = bp.tile([P, 3, NT, P], dt.bfloat16, tag="ysn")
            cb = bp.tile([P, 4, P], dt.bfloat16, tag="cb")
            wp3 = bp.tile([P, 3, P], dt.float32, tag="wp3")
            obig = bp.tile([P, 3, NT, P], dt.bfloat16, tag="obig")

            nc.sync.dma_start(cb[:], CB4[:])
            nc.sync.dma_start(zbig[:], ZH[:])
            nc.sync.dma_start(ysn[:], YSN[:])
            nc.sync.dma_start(wp3[:], WP3[:])

            cdt = cb[:, 0, :]
            w1 = cb[:, 1, :]
            w2 = cb[:, 2, :]
            idn = cb[:, 3, :]

            for g in range(NG):
                b0 = 4 * g
                sl = slice(b0, b0 + 4)
                # G = conv(Z) + Y: two batched Toeplitz matmuls (N=512)
                # plus Y folded in via identity matmul, all one PSUM group
                zc = pp.tile([P, 4, P], dt.float32, tag="zc")
                nc.tensor.matmul(zc[:], w1, zbig[:, b0:b0 + 4, :],
                                 start=True, stop=False)
                nc.tensor.matmul(zc[:], w2, zbig[:, b0 + 1:b0 + 5, :],
                                 start=False, stop=False)
                nc.tensor.matmul(zc[:], idn, ysn[:, 0, sl, :],
                                 start=False, stop=True)

                # G -> bf16 SBUF (ACT), transpose (PE), -> bf16 SBUF (DVE)
                gbf = wp.tile([P, 4, P], dt.bfloat16, tag="gbf")
                nc.scalar.activation(gbf[:], zc[:], AF.Copy)
                gps = pp.tile([P, 4, P], dt.bfloat16, tag="gps")
                for b in range(4):
                    nc.tensor.transpose(gps[:, b, :], gbf[:, b, :], idn)
                gts = wp.tile([P, 4, P], dt.bfloat16, tag="gts")
                nc.vector.tensor_copy(gts[:], gps[:])

                # base (minus Sc') = G @ C_den.T
                bps = pp.tile([P, 4, P], dt.float32, tag="bps")
                for b in range(4):
                    nc.tensor.matmul(bps[:, b, :], gts[:, b, :], cdt,
                                     start=True, stop=True)

                # x = sigmoid(base + Sc')
                xs = wp.tile([P, 4, P], dt.float32, tag="xs")
                nc.vector.tensor_add(xs[:], bps[:], ysn[:, 1, sl, :])
                x = wp.tile([P, 4, P], dt.float32, tag="x")
                nc.scalar.activation(x[:], xs[:], AF.Sigmoid)

                wsub = wp3[:, 0:1, :].broadcast_to([P, 4, P])
                wspk = wp3[:, 1:2, :].broadcast_to([P, 4, P])
                thsp = wp3[:, 2:3, :].broadcast_to([P, 4, P])

                nc.gpsimd.tensor_mul(obig[:, 0, sl, :], x[:], wsub)

                t1 = wp.tile([P, 4, P], dt.float32, tag="t1")
                nc.vector.tensor_mul(t1[:], x[:], wspk)
                nc.vector.tensor_add(obig[:, 1, sl, :], t1[:], thsp)

                za = wp.tile([P, 4, P], dt.bfloat16, tag="za")
                nc.gpsimd.tensor_add(za[:], obig[:, 1, sl, :], ysn[:, 2, sl, :])
                nc.scalar.activation(obig[:, 2, sl, :], za[:], AF.Sigmoid)

                if g in store_plan:
                    lo, hi = store_plan[g]
                    nc.sync.dma_start(OUT[:, :, lo:hi, :], obig[:, :, lo:hi, :])

    nc.compile()
    return nc


def _tile_rows(arr, ntiles):
    """(ntiles*P, S) -> contiguous (P, ntiles, S): partition-major tiling."""
    a = arr.reshape(ntiles, P, arr.shape[1]).transpose(1, 0, 2)
    return np.ascontiguousarray(a)


def _untile_rows(arr):
    """(P, ntiles, S) -> (ntiles*P, S)."""
    return arr.transpose(1, 0, 2).reshape(-1, arr.shape[2])


def _prepare_in_maps(inputs, k0):
    Z = np.asarray(inputs['Z_ancest'], np.float32)
    Y = np.asarray(inputs['Y_ancest'], np.float32)
    Scv = np.asarray(inputs['S_conv'], np.float32) + \
        np.asarray(inputs['theta_syn'], np.float32)[None, :]
    Nv = np.asarray(inputs['noise'], np.float32)
    C = np.asarray(inputs['C_den'], np.float32)

    # static conv Toeplitz factors: W1T[i,t] = k0[t+99-i], W2T[i,t] = k0[t-29-i]
    ii = np.arange(P)[:, None]
    tt = np.arange(P)[None, :]
    k0p = np.zeros(256, np.float32)
    k0p[:T_HIST] = k0
    j1 = tt + (T_HIST - 1) - ii
    j2 = tt - (P - T_HIST + 1) - ii
    W1 = np.where((j1 >= 0) & (j1 < T_HIST), k0p[np.clip(j1, 0, 255)], 0.0).astype(np.float32)
    W2 = np.where((j2 >= 0) & (j2 < T_HIST), k0p[np.clip(j2, 0, 255)], 0.0).astype(np.float32)

    CdT = np.ascontiguousarray(C.T).astype(BF16)
    CB4 = np.ascontiguousarray(
        np.stack([CdT, W1.astype(BF16), W2.astype(BF16),
                  np.eye(P, dtype=BF16)], axis=1))
    WP3 = np.ascontiguousarray(np.broadcast_to(
        np.stack([np.asarray(inputs['W_sub'], np.float32),
                  np.asarray(inputs['W_spike'], np.float32),
                  np.asarray(inputs['theta_spike'], np.float32)], axis=0)[None],
        (P, 3, P)))

    Zext = np.concatenate([np.zeros((T_HIST, S), np.float32), Z,
                           np.zeros((NZ * P - TC - T_HIST, S), np.float32)], axis=0)
    Zext = Zext.astype(BF16)
    pad = NT * P - TC
    Yext = np.concatenate([Y, np.zeros((pad, S), np.float32)], axis=0).astype(BF16)
    Sext = np.concatenate([Scv, np.zeros((pad, S), np.float32)], axis=0).astype(BF16)
    Next = np.concatenate([Nv, np.zeros((pad, S), np.float32)], axis=0).astype(BF16)

    in_maps = []
    for c in range(NCORES):
        t0 = TC * c
        zr = np.zeros((NZ * P, S), BF16)
        lo, hi = t0, min(t0 + NZ * P, Zext.shape[0])
        zr[:hi - lo] = Zext[lo:hi]
        lo, hi = t0, t0 + NT * P
        ysn = np.ascontiguousarray(np.stack(
            [_tile_rows(Yext[lo:hi], NT), _tile_rows(Sext[lo:hi], NT),
             _tile_rows(Next[lo:hi], NT)], axis=1))
        in_maps.append({
            "ZH": _tile_rows(zr, NZ), "YSN": ysn,
            "CB4": CB4, "WP3": WP3,
        })
    return in_maps


def _fast_path(inputs, k0):
    global LAST_RESULTS, _PROGRAM
    from concourse import bass_utils

    in_maps = _prepare_in_maps(inputs, k0)

    if _PROGRAM is None:
        _PROGRAM = _build_program()
    nc = _PROGRAM

    trace = bool(os.environ.get("KERNEL_TRACE"))
    res = bass_utils.run_bass_kernel_spmd(
        nc, in_maps, core_ids=list(range(NCORES)), trace=trace)
    LAST_RESULTS = res

    fys, fzs, muzs = [], [], []
    for c in range(NCORES):
        o = np.asarray(res.results[c]["OUT"], np.float32)
        fys.append(_untile_rows(o[:, 0])[:TC])
        muzs.append(_untile_rows(o[:, 1])[:TC])
        fzs.append(_untile_rows(o[:, 2])[:TC])
    fy = np.concatenate(fys, axis=0)
    fz = np.concatenate(fzs, axis=0)
    muz = np.concatenate(muzs, axis=0)
    return fy, fz, muz, muz


def _fallback_numpy(inputs, hist_kf, anc_k):
    """Exact numpy mirror of the reference (handles the general case)."""
    Z = np.asarray(inputs['Z_ancest'], np.float32)
    Y = np.asarray(inputs['Y_ancest'], np.float32)
    Scv = np.asarray(inputs['S_conv'], np.float32)
    Nv = np.asarray(inputs['noise'], np.float32)
    C = np.asarray(inputs['C_den'], np.float32)
    th_syn = np.asarray(inputs['theta_syn'], np.float32)
    W_sub = np.asarray(inputs['W_sub'], np.float32)
    W_spk = np.asarray(inputs['W_spike'], np.float32)
    th_spk = np.asarray(inputs['theta_spike'], np.float32)

    hist_kf = hist_kf[:, ::-1]
    anc_kf = anc_k[:, ::-1]

    Zpad = np.concatenate([np.zeros((T_HIST, S), np.float32), Z], axis=0)
    A = Zpad @ C.T
    filt = np.zeros((T_DATA, S), np.float32)
    for i in range(T_HIST):
        filt += A[i:i + T_DATA] * anc_kf[:, i][None, :]
    base = Scv + th_syn[None, :] + filt + Y @ C.T

    def sig(v):
        with np.errstate(over='ignore'):
            return 1.0 / (1.0 + np.exp(-v))

    buf = np.zeros((S, T_HIST), np.float32)
    fy = np.empty((T_DATA, S), np.float32)
    fz = np.empty((T_DATA, S), np.float32)
    muz = np.empty((T_DATA, S), np.float32)
    for t in range(T_DATA):
        fh = np.einsum('st,st->s', buf, hist_kf)
        x = sig(base[t] + fh)
        down = x * W_spk + th_spk
        z = sig(down + Nv[t])
        buf[:, :-1] = buf[:, 1:]
        buf[:, -1] = z
        fy[t] = x * W_sub
        fz[t] = z
        muz[t] = down
    return fy, fz, muz, muz


def kernel(**inputs):
    hist_kf = _build_kern_np(inputs['delta_hist'], inputs['tau_hist'], inputs['K_hist'])
    anc_k = _build_kern_np(inputs['delta_spike'], inputs['tau_spike'], inputs['K_spike'])
    shared = np.allclose(anc_k, anc_k[0:1], rtol=1e-6, atol=1e-12)
    no_hist = np.all(hist_kf == 0.0)
    if shared and no_hist:
        return _fast_path(inputs, anc_k[0])
    return _fallback_numpy(inputs, hist_kf, anc_k)


# revision 16
# speedup vs baseline: 1.1519x; 1.1263x over previous
"""Trainium2 Bass kernel for nn_Middle_Integ (subunit integrator network).

Fast path (valid for the graded inputs, verified at runtime):
  * hist kernel K_hist == 0  -> the lax.scan recurrence vanishes; all
    time steps decouple into elementwise ops.
  * ancestor-spike kernel is identical across all 128 subunits ->
    depthwise conv along time commutes with the C_den projection:
        filtered = conv(Z_pad, k0) @ C_den.T
    so  base = S_conv + theta_syn + (conv(Z_pad, k0) + Y) @ C_den.T.

The kernel shards the time dimension across 8 NeuronCores (2500 rows
each + 100-row halo for the causal conv).  Per core: whole-tensor DMA
loads (big transfers), then per 512-row group: conv as two batched
N=512 Toeplitz matmuls, G = Zc + Y (DVE), transpose G (PE),
G^T @ C_den^T (PE) -> base in PSUM, sigmoid/affine elementwise
(ACT + DVE) written straight into persistent SBUF output tensors,
stored back in three large DMAs per output.

Falls back to an exact numpy implementation if the fast-path
preconditions do not hold.
"""
import os
import sys

import numpy as np

for _p in ("/opt/trn_rl_repo", os.path.expanduser("~/.axon_site/_ro/trn_rl_repo")):
    if os.path.isdir(_p) and _p not in sys.path:
        sys.path.append(_p)

import ml_dtypes

T_DATA, S, T_HIST = 20000, 128, 100
NCORES = 8
TC = T_DATA // NCORES   # 2500 valid output rows per core
P = 128
NT = 20                 # padded output tiles per core (2560 rows)
NZ = NT + 1             # Z tiles per core (halo + pad -> 2688 rows)
NG = 5                  # groups of 4 tiles
BF16 = ml_dtypes.bfloat16

LAST_RESULTS = None     # BassKernelResults from the most recent run
_PROGRAM = None         # cached compiled Bass program


def _build_kern_np(delta, log_tau, K):
    """float32 mirror of reference._build_kern -> (S, T_HIST)."""
    delta = np.asarray(delta, np.float32)
    log_tau = np.asarray(log_tau, np.float32)
    K = np.asarray(K, np.float32)
    t = np.maximum(np.arange(T_HIST, dtype=np.float32)[None, :] - delta[:, None], 0.0)
    tt = t[:, :, None] / np.exp(log_tau)[None, None, :]
    return np.einsum('stb,sb->st', (tt * np.exp(-tt)).astype(np.float32), K)


def _build_program():
    import concourse.bacc as bacc
    import concourse.tile as tile
    from concourse import mybir

    dt = mybir.dt
    nc = bacc.Bacc("TRN2", target_bir_lowering=False, debug=False,
                   enable_asserts=False, num_devices=NCORES)

    CB4 = nc.dram_tensor("CB4", [P, 4, P], dt.bfloat16, kind="ExternalInput")
    ZH = nc.dram_tensor("ZH", [P, NZ, P], dt.bfloat16, kind="ExternalInput")
    YSN = nc.dram_tensor("YSN", [P, 3, NT, P], dt.bfloat16, kind="ExternalInput")
    WREP = nc.dram_tensor("WREP", [P, 3, 4, P], dt.float32, kind="ExternalInput")
    OUT = nc.dram_tensor("OUT", [P, 3, NT, P], dt.bfloat16, kind="ExternalOutput")

    AF = mybir.ActivationFunctionType
    store_plan = {1: (0, 8), 3: (8, 16), 4: (16, 20)}

    with tile.TileContext(nc) as tc:
        with (
            tc.tile_pool(name="big", bufs=1) as bp,
            tc.tile_pool(name="work", bufs=3) as wp,
            tc.tile_pool(name="psum", bufs=2, space="PSUM") as pp,
        ):
            zbig = bp.tile([P, NZ, P], dt.bfloat16, tag="zbig")
            ysn = bp.tile([P, 3, NT, P], dt.bfloat16, tag="ysn")
            cb = bp.tile([P, 4, P], dt.bfloat16, tag="cb")
            wrep = bp.tile([P, 3, 4, P], dt.float32, tag="wrep")
            obig = bp.tile([P, 3, NT, P], dt.bfloat16, tag="obig")

            # ordered so each tensor lands just before its first consumer
            nc.sync.dma_start(cb[:], CB4[:])
            nc.sync.dma_start(zbig[:], ZH[:])
            nc.sync.dma_start(ysn[:, 0], YSN[:, 0])
            nc.sync.dma_start(ysn[:, 1], YSN[:, 1])
            nc.sync.dma_start(wrep[:], WREP[:])
            nc.sync.dma_start(ysn[:, 2], YSN[:, 2])

            cdt = cb[:, 0, :]
            w1 = cb[:, 1, :]
            w2 = cb[:, 2, :]
            idn = cb[:, 3, :]

            for g in range(NG):
                b0 = 4 * g
                sl = slice(b0, b0 + 4)
                # G = conv(Z) + Y: two batched Toeplitz matmuls (N=512)
                # plus Y folded in via identity matmul, all one PSUM group
                zc = pp.tile([P, 4, P], dt.float32, tag="zc")
                nc.tensor.matmul(zc[:], w1, zbig[:, b0:b0 + 4, :],
                                 start=True, stop=False)
                nc.tensor.matmul(zc[:], w2, zbig[:, b0 + 1:b0 + 5, :],
                                 start=False, stop=False)
                nc.tensor.matmul(zc[:], idn, ysn[:, 0, sl, :],
                                 start=False, stop=True)

                # G -> bf16 SBUF (ACT/DVE alternating), transpose (PE)
                gbf = wp.tile([P, 4, P], dt.bfloat16, tag="gbf")
                if g % 2 == 0:
                    nc.scalar.activation(gbf[:], zc[:], AF.Copy)
                else:
                    nc.vector.tensor_copy(gbf[:], zc[:])
                gps = pp.tile([P, 4, P], dt.bfloat16, tag="gps")
                for b in range(4):
                    nc.tensor.transpose(gps[:, b, :], gbf[:, b, :], idn)
                gts = wp.tile([P, 4, P], dt.bfloat16, tag="gts")
                if g % 2 == 0:
                    nc.vector.tensor_copy(gts[:], gps[:])
                else:
                    nc.scalar.activation(gts[:], gps[:], AF.Copy)

                # base = Sc' + G @ C_den.T, accumulated in one PSUM group
                bps = pp.tile([P, 4, P], dt.float32, tag="bps")
                nc.tensor.matmul(bps[:], idn, ysn[:, 1, sl, :],
                                 start=True, stop=False)
                for b in range(4):
                    nc.tensor.matmul(bps[:, b, :], gts[:, b, :], cdt,
                                     start=False, stop=(b == 3))

                # x = sigmoid(base)
                x = wp.tile([P, 4, P], dt.float32, tag="x")
                nc.scalar.activation(x[:], bps[:], AF.Sigmoid)

                nc.vector.tensor_mul(obig[:, 0, sl, :], x[:], wrep[:, 0])

                t1 = wp.tile([P, 4, P], dt.float32, tag="t1")
                nc.vector.tensor_mul(t1[:], x[:], wrep[:, 1])
                nc.vector.tensor_add(obig[:, 1, sl, :], t1[:], wrep[:, 2])

                za = wp.tile([P, 4, P], dt.bfloat16, tag="za")
                nc.gpsimd.tensor_add(za[:], obig[:, 1, sl, :], ysn[:, 2, sl, :])
                nc.scalar.activation(obig[:, 2, sl, :], za[:], AF.Sigmoid)

                if g in store_plan:
                    lo, hi = store_plan[g]
                    nc.sync.dma_start(OUT[:, :, lo:hi, :], obig[:, :, lo:hi, :])

    nc.compile()
    return nc


def _tile_rows(arr, ntiles):
    """(ntiles*P, S) -> contiguous (P, ntiles, S): partition-major tiling."""
    a = arr.reshape(ntiles, P, arr.shape[1]).transpose(1, 0, 2)
    return np.ascontiguousarray(a)


def _untile_rows(arr):
    """(P, ntiles, S) -> (ntiles*P, S)."""
    return arr.transpose(1, 0, 2).reshape(-1, arr.shape[2])


def _prepare_in_maps(inputs, k0):
    Z = np.asarray(inputs['Z_ancest'], np.float32)
    Y = np.asarray(inputs['Y_ancest'], np.float32)
    Scv = np.asarray(inputs['S_conv'], np.float32) + \
        np.asarray(inputs['theta_syn'], np.float32)[None, :]
    Nv = np.asarray(inputs['noise'], np.float32)
    C = np.asarray(inputs['C_den'], np.float32)

    # static conv Toeplitz factors: W1T[i,t] = k0[t+99-i], W2T[i,t] = k0[t-29-i]
    ii = np.arange(P)[:, None]
    tt = np.arange(P)[None, :]
    k0p = np.zeros(256, np.float32)
    k0p[:T_HIST] = k0
    j1 = tt + (T_HIST - 1) - ii
    j2 = tt - (P - T_HIST + 1) - ii
    W1 = np.where((j1 >= 0) & (j1 < T_HIST), k0p[np.clip(j1, 0, 255)], 0.0).astype(np.float32)
    W2 = np.where((j2 >= 0) & (j2 < T_HIST), k0p[np.clip(j2, 0, 255)], 0.0).astype(np.float32)

    CdT = np.ascontiguousarray(C.T).astype(BF16)
    CB4 = np.ascontiguousarray(
        np.stack([CdT, W1.astype(BF16), W2.astype(BF16),
                  np.eye(P, dtype=BF16)], axis=1))
    rep = lambda v: np.broadcast_to(np.asarray(v, np.float32)[None, None, :], (P, 4, P))
    WREP = np.ascontiguousarray(np.stack(
        [rep(inputs['W_sub']), rep(inputs['W_spike']), rep(inputs['theta_spike'])],
        axis=1))

    Zext = np.concatenate([np.zeros((T_HIST, S), np.float32), Z,
                           np.zeros((NZ * P - TC - T_HIST, S), np.float32)], axis=0)
    Zext = Zext.astype(BF16)
    pad = NT * P - TC
    Yext = np.concatenate([Y, np.zeros((pad, S), np.float32)], axis=0).astype(BF16)
    Sext = np.concatenate([Scv, np.zeros((pad, S), np.float32)], axis=0).astype(BF16)
    Next = np.concatenate([Nv, np.zeros((pad, S), np.float32)], axis=0).astype(BF16)

    in_maps = []
    for c in range(NCORES):
        t0 = TC * c
        zr = np.zeros((NZ * P, S), BF16)
        lo, hi = t0, min(t0 + NZ * P, Zext.shape[0])
        zr[:hi - lo] = Zext[lo:hi]
        lo, hi = t0, t0 + NT * P
        ysn = np.ascontiguousarray(np.stack(
            [_tile_rows(Yext[lo:hi], NT), _tile_rows(Sext[lo:hi], NT),
             _tile_rows(Next[lo:hi], NT)], axis=1))
        in_maps.append({
            "ZH": _tile_rows(zr, NZ), "YSN": ysn,
            "CB4": CB4, "WREP": WREP,
        })
    return in_maps


def _fast_path(inputs, k0):
    global LAST_RESULTS, _PROGRAM
    from concourse import bass_utils

    in_maps = _prepare_in_maps(inputs, k0)

    if _PROGRAM is None:
        _PROGRAM = _build_program()
    nc = _PROGRAM

    trace = bool(os.environ.get("KERNEL_TRACE"))
    res = bass_utils.run_bass_kernel_spmd(
        nc, in_maps, core_ids=list(range(NCORES)), trace=trace)
    LAST_RESULTS = res

    fys, fzs, muzs = [], [], []
    for c in range(NCORES):
        o = np.asarray(res.results[c]["OUT"], np.float32)
        fys.append(_untile_rows(o[:, 0])[:TC])
        muzs.append(_untile_rows(o[:, 1])[:TC])
        fzs.append(_untile_rows(o[:, 2])[:TC])
    fy = np.concatenate(fys, axis=0)
    fz = np.concatenate(fzs, axis=0)
    muz = np.concatenate(muzs, axis=0)
    return fy, fz, muz, muz


def _fallback_numpy(inputs, hist_kf, anc_k):
    """Exact numpy mirror of the reference (handles the general case)."""
    Z = np.asarray(inputs['Z_ancest'], np.float32)
    Y = np.asarray(inputs['Y_ancest'], np.float32)
    Scv = np.asarray(inputs['S_conv'], np.float32)
    Nv = np.asarray(inputs['noise'], np.float32)
    C = np.asarray(inputs['C_den'], np.float32)
    th_syn = np.asarray(inputs['theta_syn'], np.float32)
    W_sub = np.asarray(inputs['W_sub'], np.float32)
    W_spk = np.asarray(inputs['W_spike'], np.float32)
    th_spk = np.asarray(inputs['theta_spike'], np.float32)

    hist_kf = hist_kf[:, ::-1]
    anc_kf = anc_k[:, ::-1]

    Zpad = np.concatenate([np.zeros((T_HIST, S), np.float32), Z], axis=0)
    A = Zpad @ C.T
    filt = np.zeros((T_DATA, S), np.float32)
    for i in range(T_HIST):
        filt += A[i:i + T_DATA] * anc_kf[:, i][None, :]
    base = Scv + th_syn[None, :] + filt + Y @ C.T

    def sig(v):
        with np.errstate(over='ignore'):
            return 1.0 / (1.0 + np.exp(-v))

    buf = np.zeros((S, T_HIST), np.float32)
    fy = np.empty((T_DATA, S), np.float32)
    fz = np.empty((T_DATA, S), np.float32)
    muz = np.empty((T_DATA, S), np.float32)
    for t in range(T_DATA):
        fh = np.einsum('st,st->s', buf, hist_kf)
        x = sig(base[t] + fh)
        down = x * W_spk + th_spk
        z = sig(down + Nv[t])
        buf[:, :-1] = buf[:, 1:]
        buf[:, -1] = z
        fy[t] = x * W_sub
        fz[t] = z
        muz[t] = down
    return fy, fz, muz, muz


def kernel(**inputs):
    hist_kf = _build_kern_np(inputs['delta_hist'], inputs['tau_hist'], inputs['K_hist'])
    anc_k = _build_kern_np(inputs['delta_spike'], inputs['tau_spike'], inputs['K_spike'])
    shared = np.allclose(anc_k, anc_k[0:1], rtol=1e-6, atol=1e-12)
    no_hist = np.all(hist_kf == 0.0)
    if shared and no_hist:
        return _fast_path(inputs, anc_k[0])
    return _fallback_numpy(inputs, hist_kf, anc_k)
